# revision 8
# baseline (speedup 1.0000x reference)
"""Trainium2 Bass kernel for nn_ConcatenateAttention.

Per batch b:
    n4 = w42[:, :D] @ keys[b] + (w42[:, D:] @ query[b] + b4)   # [H, T]
    a4 = tanh(n4)
    n5 = w54 @ a4  (+ b5, dropped: softmax is shift-invariant)  # [1, T]
    a5 = softmax(n5)  over T   (no max subtraction needed: |n5| <= sum|w54| ~ 4)
    a6 = values[b] @ a5                                         # [D, 1]

Sharding: batch B=32 across 8 cores (pure data parallel), params replicated.

On-core dataflow (per local batch, per 512-wide t-chunk):
  - PE: n4 = accumulate 4 K-chunks of w42aT x keys       (float32r fast path)
  - ACT: a4 = tanh(n4 + qb)  with qb = w42bT@q + b4 as per-partition bias
  - PE: n5 = accumulate 4 h-chunks of w54T x a4
  - ACT: e5 = exp(n5), accum_out -> softmax denominator partial
  - PE: broadcast e5 row to 128 partitions (ones x e5)
  - DVE: tensor_tensor_reduce(values_tile * e5_bcast) -> a6 partials
  Finally a6 = partials_sum * (1/Z) and DMA out.
"""

import numpy as np

B, D, H, T = 32, 512, 512, 4096
NCORES = 8
BL = B // NCORES            # batches per core
P = 128
KC = D // P                 # contraction chunks (d)
HT = H // P                 # h tiles
DT = D // P                 # d tiles for values
TW = 1024                   # t-chunk width (DMA/DVE granularity)
TCH = T // TW               # t-chunks
NH = TW // 512              # 512-wide matmul halves per chunk

TRACE = False               # set by test.py for profiling runs
TRACE_DIR = None            # set by test.py; keeps NTFF/perfetto artifacts
LAST_RESULTS = None         # BassKernelResults of the last run

_NC = None


def _build_nc():
    from contextlib import ExitStack

    import concourse.bass as bass  # noqa: F401
    import concourse.tile as tile
    from concourse import bacc, mybir

    f32 = mybir.dt.float32
    f32r = mybir.dt.float32r
    bf16 = mybir.dt.bfloat16
    TANH = mybir.ActivationFunctionType.Tanh
    EXP = mybir.ActivationFunctionType.Exp
    MULT = mybir.AluOpType.mult
    ADD = mybir.AluOpType.add
    AX = mybir.AxisListType.X

    nc = bacc.Bacc("TRN2", target_bir_lowering=False, debug=False)

    keys_d = nc.dram_tensor("keys_loc", [BL, D, T], bf16, kind="ExternalInput")
    vals_d = nc.dram_tensor("vals_loc", [BL, D, T], f32, kind="ExternalInput")
    wa_d = nc.dram_tensor("wa_p", [P, KC, H], bf16, kind="ExternalInput")
    wb_d = nc.dram_tensor("wb_p", [P, KC, H], f32, kind="ExternalInput")
    w54_d = nc.dram_tensor("w54_p", [P, HT], bf16, kind="ExternalInput")
    b4_d = nc.dram_tensor("b4_p", [P, HT], f32, kind="ExternalInput")
    q_d = nc.dram_tensor("q_p", [P, KC, BL], f32, kind="ExternalInput")
    out_d = nc.dram_tensor("out_t", [DT, P, BL], f32, kind="ExternalOutput")

    keys_ap = keys_d.ap().rearrange("b (kc p) t -> b p kc t", p=P)
    vals_ap = vals_d.ap().rearrange("b (dt p) t -> b p dt t", p=P)

    with tile.TileContext(nc) as tc, ExitStack() as ctx:
        singles = ctx.enter_context(tc.tile_pool(name="singles", bufs=1))
        kv = ctx.enter_context(tc.tile_pool(name="kv", bufs=3))
        work = ctx.enter_context(tc.tile_pool(name="work", bufs=2))
        ps2 = ctx.enter_context(tc.tile_pool(name="ps2", bufs=2, space="PSUM"))
        ps1 = ctx.enter_context(tc.tile_pool(name="ps1", bufs=1, space="PSUM"))

        wa = singles.tile([P, KC, H], bf16)
        nc.sync.dma_start(out=wa, in_=wa_d.ap())
        wb = singles.tile([P, KC, H], f32)
        nc.sync.dma_start(out=wb, in_=wb_d.ap())
        w54t = singles.tile([P, HT], bf16)
        nc.sync.dma_start(out=w54t, in_=w54_d.ap())
        b4t = singles.tile([P, HT], f32)
        nc.sync.dma_start(out=b4t, in_=b4_d.ap())
        qt = singles.tile([P, KC, BL], f32)
        nc.sync.dma_start(out=qt, in_=q_d.ap())
        ones_f = singles.tile([1, P], f32)
        nc.vector.memset(ones_f, 1.0)
        ones_t = singles.tile([1, P], f32r)
        nc.vector.tensor_copy(ones_t, ones_f)

        # qb[h, b] = (w42b @ q)[h, b] + b4[h]  -- per-partition bias for tanh
        qb_s = singles.tile([P, HT, BL], f32)
        for ht in range(HT):
            qbp = ps1.tile([P, BL], f32, tag="smallp")
            for kc in range(KC):
                nc.tensor.matmul(
                    qbp,
                    lhsT=wb[:, kc, ht * P:(ht + 1) * P],
                    rhs=qt[:, kc, :],
                    start=(kc == 0),
                    stop=(kc == KC - 1),
                )
            nc.vector.tensor_scalar_add(
                out=qb_s[:, ht, :], in0=qbp, scalar1=b4t[:, ht:ht + 1]
            )

        a6u = singles.tile([P, DT, BL], f32)
        zsum = singles.tile([1, BL], f32)

        for b in range(BL):
            a6p = work.tile([P, DT, TCH], f32, tag="a6p")
            zp = work.tile([1, TCH], f32, tag="zp")
            for tci in range(TCH):
                tsl = slice(tci * TW, (tci + 1) * TW)
                kt = kv.tile([P, KC, TW], bf16, tag="keys")
                for kc in range(KC):
                    nc.sync.dma_start(
                        out=kt[:, kc, :], in_=keys_ap[b][:, kc, tsl]
                    )
                vt = kv.tile([P, DT, TW], f32, tag="vals")
                for dt_ in range(DT):
                    nc.sync.dma_start(
                        out=vt[:, dt_, :], in_=vals_ap[b][:, dt_, tsl]
                    )

                n5p = ps1.tile([1, TW], f32, tag="n5")
                for half in range(NH):
                    hsl = slice(half * 512, (half + 1) * 512)
                    a4 = work.tile([P, HT, 512], bf16, tag="a4")
                    for ht in range(HT):
                        n4p = ps2.tile([P, 512], f32, tag="n4")
                        for kc in range(KC):
                            nc.tensor.matmul(
                                n4p,
                                lhsT=wa[:, kc, ht * P:(ht + 1) * P],
                                rhs=kt[:, kc, hsl],
                                start=(kc == 0),
                                stop=(kc == KC - 1),
                            )
                        nc.scalar.activation(
                            out=a4[:, ht, :],
                            in_=n4p,
                            func=TANH,
                            bias=qb_s[:, ht, b:b + 1],
                            scale=1.0,
                        )
                    for ht in range(HT):
                        nc.tensor.matmul(
                            n5p[:, hsl],
                            lhsT=w54t[:, ht:ht + 1],
                            rhs=a4[:, ht, :],
                            start=(ht == 0),
                            stop=(ht == HT - 1),
                        )

                e5 = work.tile([1, TW], f32r, tag="e5")
                nc.scalar.activation(
                    out=e5, in_=n5p, func=EXP, accum_out=zp[:, tci:tci + 1]
                )

                eb = ps1.tile([P, TW], f32, tag="eb")
                for half in range(NH):
                    hsl = slice(half * 512, (half + 1) * 512)
                    nc.tensor.matmul(
                        eb[:, hsl],
                        lhsT=ones_t,
                        rhs=e5[:, hsl],
                        start=True,
                        stop=True,
                    )

                for dt_ in range(DT):
                    prod = work.tile([P, TW], f32, tag="prod")
                    nc.vector.scalar_tensor_tensor(
                        out=prod,
                        in0=vt[:, dt_, :],
                        scalar=1.0,
                        in1=eb,
                        op0=MULT,
                        op1=MULT,
                        accum_out=a6p[:, dt_, tci:tci + 1],
                    )

            nc.vector.tensor_reduce(out=a6u[:, :, b], in_=a6p, axis=AX, op=ADD)
            nc.vector.tensor_reduce(out=zsum[:, b:b + 1], in_=zp, axis=AX, op=ADD)

        zr = singles.tile([1, BL], f32)
        nc.vector.reciprocal(zr, zsum)
        zb = ps1.tile([P, BL], f32, tag="smallp")
        nc.tensor.matmul(zb, lhsT=ones_f, rhs=zr, start=True, stop=True)
        for dt_ in range(DT):
            a6f = work.tile([P, BL], f32, tag="a6f")
            nc.vector.tensor_mul(a6f, a6u[:, dt_, :], zb)
            nc.sync.dma_start(out=out_d.ap()[dt_], in_=a6f)

    nc.compile()
    return nc


def get_nc():
    global _NC
    if _NC is None:
        _NC = _build_nc()
    return _NC


def make_in_maps(query, keys, values, w42, b4, w54):
    """Host-side packing (layout only) + per-core sharding."""
    import ml_dtypes

    bf = ml_dtypes.bfloat16
    f = np.float32
    w42aT = np.ascontiguousarray(w42[:, :D].T, dtype=f)   # [D, H]
    w42bT = np.ascontiguousarray(w42[:, D:].T, dtype=f)   # [D, H]
    wa_p = np.ascontiguousarray(w42aT.reshape(KC, P, H).transpose(1, 0, 2)).astype(bf)
    wb_p = np.ascontiguousarray(w42bT.reshape(KC, P, H).transpose(1, 0, 2))
    w54_p = np.ascontiguousarray(w54.reshape(HT, P).T, dtype=f).astype(bf)  # [P, HT]
    b4_p = np.ascontiguousarray(b4[:, 0].reshape(HT, P).T, dtype=f)  # [P, HT]

    in_maps = []
    for c in range(NCORES):
        sl = slice(c * BL, (c + 1) * BL)
        q_loc = np.asarray(query[sl, :, 0], dtype=f)                 # [BL, D]
        q_p = np.ascontiguousarray(q_loc.T.reshape(KC, P, BL).transpose(1, 0, 2))
        in_maps.append(
            {
                "keys_loc": np.ascontiguousarray(keys[sl], dtype=f).astype(bf),
                "vals_loc": np.ascontiguousarray(values[sl], dtype=f),
                "wa_p": wa_p,
                "wb_p": wb_p,
                "w54_p": w54_p,
                "b4_p": b4_p,
                "q_p": q_p,
            }
        )
    return in_maps


def gather_out(results):
    """results: list of {\"out_t\": [DT, P, BL]} per core -> [B, D, 1] fp32."""
    outs = []
    for c in range(NCORES):
        ot = results[c]["out_t"]                       # [DT, P, BL]
        outs.append(ot.transpose(2, 0, 1).reshape(BL, D))
    return np.concatenate(outs, axis=0)[:, :, None].astype(np.float32)


def kernel(query, keys, values, w42, b4, w54, b5):
    global LAST_RESULTS
    from concourse import bass_utils

    nc = get_nc()
    in_maps = make_in_maps(query, keys, values, w42, b4, w54)
    res = bass_utils.run_bass_kernel_spmd(
        nc, in_maps, core_ids=list(range(NCORES)), trace=TRACE, tmpdir=TRACE_DIR
    )
    LAST_RESULTS = res
    return gather_out(res.results)


# revision 12
# speedup vs baseline: 1.1667x; 1.1667x over previous
"""Trainium2 Bass kernel for nn_ConcatenateAttention.

Per batch b:
    n4 = w42[:, :D] @ keys[b] + (w42[:, D:] @ query[b] + b4)   # [H, T]
    a4 = tanh(n4)
    n5 = w54 @ a4  (+ b5, dropped: softmax is shift-invariant)  # [1, T]
    a5 = softmax(n5)  over T   (no max subtraction needed: |n5| <= sum|w54| ~ 4)
    a6 = values[b] @ a5                                         # [D, 1]

Sharding: batch B=32 across 8 cores (pure data parallel), params replicated.

On-core dataflow (per local batch, per 512-wide t-chunk):
  - PE: n4 = accumulate 4 K-chunks of w42aT x keys       (float32r fast path)
  - ACT: a4 = tanh(n4 + qb)  with qb = w42bT@q + b4 as per-partition bias
  - PE: n5 = accumulate 4 h-chunks of w54T x a4
  - ACT: e5 = exp(n5), accum_out -> softmax denominator partial
  - PE: broadcast e5 row to 128 partitions (ones x e5)
  - DVE: tensor_tensor_reduce(values_tile * e5_bcast) -> a6 partials
  Finally a6 = partials_sum * (1/Z) and DMA out.
"""

import numpy as np

B, D, H, T = 32, 512, 512, 4096
NCORES = 8
BL = B // NCORES            # batches per core
P = 128
KC = D // P                 # contraction chunks (d)
HT = H // P                 # h tiles
DT = D // P                 # d tiles for values
TW = 512                    # t-chunk width
TCH = T // TW               # t-chunks

TRACE = False               # set by test.py for profiling runs
TRACE_DIR = None            # set by test.py; keeps NTFF/perfetto artifacts
LAST_RESULTS = None         # BassKernelResults of the last run

_NC = None


def _build_nc():
    from contextlib import ExitStack

    import concourse.bass as bass  # noqa: F401
    import concourse.tile as tile
    from concourse import bacc, mybir

    f32 = mybir.dt.float32
    f32r = mybir.dt.float32r
    bf16 = mybir.dt.bfloat16
    TANH = mybir.ActivationFunctionType.Tanh
    EXP = mybir.ActivationFunctionType.Exp
    MULT = mybir.AluOpType.mult
    ADD = mybir.AluOpType.add
    AX = mybir.AxisListType.X

    nc = bacc.Bacc("TRN2", target_bir_lowering=False, debug=False)

    keys_d = nc.dram_tensor("keys_loc", [BL, D, T], bf16, kind="ExternalInput")
    vals_d = nc.dram_tensor("vals_loc", [BL, D, T], f32, kind="ExternalInput")
    wa_d = nc.dram_tensor("wa_p", [P, KC, H], bf16, kind="ExternalInput")
    wb_d = nc.dram_tensor("wb_p", [P, KC, H], f32, kind="ExternalInput")
    w54_d = nc.dram_tensor("w54_p", [P, HT], bf16, kind="ExternalInput")
    b4_d = nc.dram_tensor("b4_p", [P, HT], f32, kind="ExternalInput")
    q_d = nc.dram_tensor("q_p", [P, KC, BL], f32, kind="ExternalInput")
    out_d = nc.dram_tensor("out_t", [DT, P, BL], f32, kind="ExternalOutput")

    keys_ap = keys_d.ap().rearrange("b (kc p) t -> b p kc t", p=P)
    vals_ap = vals_d.ap().rearrange("b (dt p) t -> b p dt t", p=P)

    with tile.TileContext(nc) as tc, ExitStack() as ctx:
        singles = ctx.enter_context(tc.tile_pool(name="singles", bufs=1))
        kv = ctx.enter_context(tc.tile_pool(name="kv", bufs=3))
        work = ctx.enter_context(tc.tile_pool(name="work", bufs=2))
        ps2 = ctx.enter_context(tc.tile_pool(name="ps2", bufs=2, space="PSUM"))
        ps1 = ctx.enter_context(tc.tile_pool(name="ps1", bufs=1, space="PSUM"))

        wa = singles.tile([P, KC, H], bf16)
        for kc in range(KC):
            nc.sync.dma_start(out=wa[:, kc, :], in_=wa_d.ap()[:, kc, :])
        wb = singles.tile([P, KC, H], f32)
        for kc in range(KC):
            nc.sync.dma_start(out=wb[:, kc, :], in_=wb_d.ap()[:, kc, :])
        w54t = singles.tile([P, HT], bf16)
        nc.sync.dma_start(out=w54t, in_=w54_d.ap())
        b4t = singles.tile([P, HT], f32)
        nc.sync.dma_start(out=b4t, in_=b4_d.ap())
        qt = singles.tile([P, KC, BL], f32)
        nc.sync.dma_start(out=qt, in_=q_d.ap())
        ones_f = singles.tile([1, P], f32)
        nc.vector.memset(ones_f, 1.0)
        ones_t = singles.tile([1, P], f32r)
        nc.vector.tensor_copy(ones_t, ones_f)

        # qb[h, b] = (w42b @ q)[h, b] + b4[h]  -- per-partition bias for tanh
        qb_s = singles.tile([P, HT, BL], f32)
        for ht in range(HT):
            qbp = ps1.tile([P, BL], f32, tag="smallp")
            for kc in range(KC):
                nc.tensor.matmul(
                    qbp,
                    lhsT=wb[:, kc, ht * P:(ht + 1) * P],
                    rhs=qt[:, kc, :],
                    start=(kc == 0),
                    stop=(kc == KC - 1),
                )
            nc.vector.tensor_scalar_add(
                out=qb_s[:, ht, :], in0=qbp, scalar1=b4t[:, ht:ht + 1]
            )

        a6u = singles.tile([P, DT, BL], f32)
        zsum = singles.tile([1, BL], f32)

        for b in range(BL):
            a6p = work.tile([P, DT, TCH], f32, tag="a6p")
            zp = work.tile([1, TCH], f32, tag="zp")
            for tci in range(TCH):
                tsl = slice(tci * TW, (tci + 1) * TW)
                first = b == 0 and tci == 0
                kt = kv.tile([P, KC, TW], bf16, tag="keys")
                if first:
                    for kc in range(KC):
                        nc.sync.dma_start(
                            out=kt[:, kc, :], in_=keys_ap[b][:, kc, tsl]
                        )
                else:
                    nc.sync.dma_start(out=kt, in_=keys_ap[b][:, :, tsl])
                vt = kv.tile([P, DT, TW], f32, tag="vals")
                if first:
                    for dt_ in range(DT):
                        nc.sync.dma_start(
                            out=vt[:, dt_, :], in_=vals_ap[b][:, dt_, tsl]
                        )
                else:
                    nc.sync.dma_start(out=vt, in_=vals_ap[b][:, :, tsl])

                a4 = work.tile([P, HT, TW], bf16, tag="a4")
                for ht in range(HT):
                    n4p = ps2.tile([P, TW], f32, tag="n4")
                    for kc in range(KC):
                        nc.tensor.matmul(
                            n4p,
                            lhsT=wa[:, kc, ht * P:(ht + 1) * P],
                            rhs=kt[:, kc, :],
                            start=(kc == 0),
                            stop=(kc == KC - 1),
                        )
                    nc.scalar.activation(
                        out=a4[:, ht, :],
                        in_=n4p,
                        func=TANH,
                        bias=qb_s[:, ht, b:b + 1],
                        scale=1.0,
                    )

                n5p = ps2.tile([1, TW], f32, tag="n5")
                for ht in range(HT):
                    nc.tensor.matmul(
                        n5p,
                        lhsT=w54t[:, ht:ht + 1],
                        rhs=a4[:, ht, :],
                        start=(ht == 0),
                        stop=(ht == HT - 1),
                    )

                e5 = work.tile([1, TW], f32r, tag="e5")
                nc.scalar.activation(
                    out=e5, in_=n5p, func=EXP, accum_out=zp[:, tci:tci + 1]
                )

                ebb = ps2.tile([P, TW], f32, tag="eb")
                nc.tensor.matmul(ebb, lhsT=ones_t, rhs=e5, start=True, stop=True)

                for dt_ in range(DT):
                    prod = work.tile([P, TW], f32, tag="prod")
                    nc.vector.scalar_tensor_tensor(
                        out=prod,
                        in0=vt[:, dt_, :],
                        scalar=1.0,
                        in1=ebb,
                        op0=MULT,
                        op1=MULT,
                        accum_out=a6p[:, dt_, tci:tci + 1],
                    )

            nc.vector.tensor_reduce(out=a6u[:, :, b], in_=a6p, axis=AX, op=ADD)
            nc.vector.tensor_reduce(out=zsum[:, b:b + 1], in_=zp, axis=AX, op=ADD)

        zr = singles.tile([1, BL], f32)
        nc.vector.reciprocal(zr, zsum)
        zb = ps1.tile([P, BL], f32, tag="smallp")
        nc.tensor.matmul(zb, lhsT=ones_f, rhs=zr, start=True, stop=True)
        for dt_ in range(DT):
            a6f = work.tile([P, BL], f32, tag="a6f")
            nc.vector.tensor_mul(a6f, a6u[:, dt_, :], zb)
            nc.sync.dma_start(out=out_d.ap()[dt_], in_=a6f)

    nc.compile()
    return nc


def get_nc():
    global _NC
    if _NC is None:
        _NC = _build_nc()
    return _NC


def make_in_maps(query, keys, values, w42, b4, w54):
    """Host-side packing (layout only) + per-core sharding."""
    import ml_dtypes

    bf = ml_dtypes.bfloat16
    f = np.float32
    w42aT = np.ascontiguousarray(w42[:, :D].T, dtype=f)   # [D, H]
    w42bT = np.ascontiguousarray(w42[:, D:].T, dtype=f)   # [D, H]
    wa_p = np.ascontiguousarray(w42aT.reshape(KC, P, H).transpose(1, 0, 2)).astype(bf)
    wb_p = np.ascontiguousarray(w42bT.reshape(KC, P, H).transpose(1, 0, 2))
    w54_p = np.ascontiguousarray(w54.reshape(HT, P).T, dtype=f).astype(bf)  # [P, HT]
    b4_p = np.ascontiguousarray(b4[:, 0].reshape(HT, P).T, dtype=f)  # [P, HT]

    in_maps = []
    for c in range(NCORES):
        sl = slice(c * BL, (c + 1) * BL)
        q_loc = np.asarray(query[sl, :, 0], dtype=f)                 # [BL, D]
        q_p = np.ascontiguousarray(q_loc.T.reshape(KC, P, BL).transpose(1, 0, 2))
        in_maps.append(
            {
                "keys_loc": np.ascontiguousarray(keys[sl], dtype=f).astype(bf),
                "vals_loc": np.ascontiguousarray(values[sl], dtype=f),
                "wa_p": wa_p,
                "wb_p": wb_p,
                "w54_p": w54_p,
                "b4_p": b4_p,
                "q_p": q_p,
            }
        )
    return in_maps


def gather_out(results):
    """results: list of {\"out_t\": [DT, P, BL]} per core -> [B, D, 1] fp32."""
    outs = []
    for c in range(NCORES):
        ot = results[c]["out_t"]                       # [DT, P, BL]
        outs.append(ot.transpose(2, 0, 1).reshape(BL, D))
    return np.concatenate(outs, axis=0)[:, :, None].astype(np.float32)


def kernel(query, keys, values, w42, b4, w54, b5):
    global LAST_RESULTS
    from concourse import bass_utils

    nc = get_nc()
    in_maps = make_in_maps(query, keys, values, w42, b4, w54)
    res = bass_utils.run_bass_kernel_spmd(
        nc, in_maps, core_ids=list(range(NCORES)), trace=TRACE, tmpdir=TRACE_DIR
    )
    LAST_RESULTS = res
    return gather_out(res.results)
